# revision 1
# baseline (speedup 1.0000x reference)
"""Trainium2 Bass kernel for nn_Encoder_P: unwrap-diff-square front-end + 4 dilated
convs with dense concatenation, fused end-to-end on-chip.

Strategy (pure data parallel, 1 batch sample per NeuronCore, 8 cores):
  - The unwrap/diff/pad chain collapses: cumsum cancels in the diff, so
    sq[h] = wrap(p[h] - p[h-1])^2 (row 0 = 0), wrap(v) = v - 2*pi*k with
    k = (v>=pi) + (v>=3pi) - (v<=-pi) - (v<=-3pi).
  - Duplicate concat channels are folded into effective conv weights
    (conv3: 8->7 input planes, conv4: 20->15).
  - Each conv runs on TensorE as banded matmuls over the H (partition) axis:
    lhsT is a banded [128,128] H-shift matrix built on-device (DVE) from 5
    shared shifted-identity masters scaled by runtime weight scalars; rhs is
    the input plane tile [128 H, 516 Wpad]; PSUM accumulates over (ci, kw).
  - Planes are stored as 5 overlapping H-tiles (stride 104, halo 12) of
    [128, 516] with zeroed W margins, so conv H/W reach never crosses a tile.
"""

import numpy as np

import concourse.bacc as bacc
import concourse.bass as bass
import concourse.mybir as mybir
import concourse.tile as tile
from concourse import bass_utils

F32 = mybir.dt.float32
MM_DT = mybir.dt.float32r  # full-rate fp32 matmul path (1 cyc/row at N>=256)
DEFAULT_MM = "f32r"  # flip to "bf16" only with HW-validated accuracy+speed

H = 512
W = 512
S = 107          # tile stride in rows (chosen so 512-(S*4-HALO) == 96, a legal
                 # compute-op partition start for the bottom edge-zero memset)
HALO = 12        # halo rows above/below each tile
NT = 5           # number of H tiles
WPAD = 516       # 2 zero cols + 512 + 2 zero cols
P = 128
PI = float(np.pi)

# conv specs: (dil, pad_top, pad_left, KH, KW)
CONV_GEOM = [
    (1, 1, 1, 4, 4),   # conv1: 4x4 dil1, 'same' pad (1,2)
    (2, 2, 2, 3, 3),   # conv2: 3x3 dil2, pad (2,2)
    (3, 1, 1, 2, 2),   # conv3: 2x2 dil3, pad (1,2)
    (4, 0, 0, 1, 1),   # conv4: 1x1
]

PLANE_NAMES = (
    ["sq", "c1_0", "c1_1"]
    + [f"c2_{i}" for i in range(4)]
    + [f"c3_{i}" for i in range(8)]
)
CONV_INPUTS = [
    ["sq"],
    ["c1_0", "c1_1", "sq"],
    [f"c2_{i}" for i in range(4)] + ["c1_0", "c1_1", "sq"],
    [f"c3_{i}" for i in range(8)] + [f"c2_{i}" for i in range(4)]
    + ["c1_0", "c1_1", "sq"],
]
CONV_OUT = [2, 4, 8, 16]
DELTAS = [-2, -1, 0, 1, 2]  # identity master shifts

# output channel -> source plane ("c4_o" channels handled separately)
CH_MAP = (
    [f"c4_{i}" for i in range(16)]
    + [f"c3_{i}" for i in range(8)]
    + [f"c2_{i}" for i in range(4)]
    + ["c1_0", "c1_1", "sq", "sq", "c1_0", "c1_1", "sq", "sq"]
    + [f"c2_{i}" for i in range(4)]
    + ["c1_0", "c1_1", "sq", "sq"]
    + ["c1_0", "c1_1", "sq", "sq"]
)

NSCAL = sum(
    CONV_OUT[c] * len(CONV_INPUTS[c]) * CONV_GEOM[c][3] * CONV_GEOM[c][4]
    for c in range(4)
)  # 604


def _fold_weights(w1, w2, w3, w4):
    w3f = np.zeros((8, 7, 2, 2), np.float32)
    w3f[:, :6] = w3[:, :6]
    w3f[:, 6] = w3[:, 6] + w3[:, 7]
    w4f = np.zeros((16, 15, 1, 1), np.float32)
    w4f[:, :12] = w4[:, :12]
    w4f[:, 12] = w4[:, 12] + w4[:, 16]
    w4f[:, 13] = w4[:, 13] + w4[:, 17]
    w4f[:, 14] = w4[:, 14] + w4[:, 15] + w4[:, 18] + w4[:, 19]
    return [w1.astype(np.float32), w2.astype(np.float32), w3f, w4f]


def _host_tables(inputs):
    """wtab [128, NSCAL], ident [5*128, 128], bias [128, 30] host arrays."""
    wf = _fold_weights(inputs["w1"], inputs["w2"], inputs["w3"], inputs["w4"])
    scal = []
    for c in range(4):
        dil, pad_top, _, KH, KW = CONV_GEOM[c]
        for o in range(CONV_OUT[c]):
            for ci in range(len(CONV_INPUTS[c])):
                for kw in range(KW):
                    for kh in range(KH):
                        scal.append(wf[c][o, ci, kh, kw])
    assert len(scal) == NSCAL
    wtab = np.tile(np.asarray(scal, np.float32)[None, :], (P, 1))
    ident = np.concatenate(
        [np.eye(P, dtype=np.float32, k=-d) for d in DELTAS], axis=0
    )
    bias = np.concatenate(
        [inputs["b1"], inputs["b2"], inputs["b3"], inputs["b4"]]
    ).astype(np.float32)
    bias = np.tile(bias[None, :], (P, 1))
    return wtab, ident, bias


def build_nc(loop_k=1, out_mode='full', skip_bands=False, mm='f32r'):
    nc = bacc.Bacc("TRN2", target_bir_lowering=False, debug=False)
    mm_dt = mybir.dt.bfloat16 if mm == 'bf16' else MM_DT

    def msafe(ap):
        # memset target: walrus rejects float32r memsets; bitcast those to f32
        return ap.bitcast(F32) if mm != 'bf16' else ap

    p_dram = nc.dram_tensor("p", [H, W], F32, kind="ExternalInput")
    ident_dram = nc.dram_tensor("ident", [5 * P, P], F32, kind="ExternalInput")
    wtab_dram = nc.dram_tensor("wtab", [P, NSCAL], F32, kind="ExternalInput")
    bias_dram = nc.dram_tensor("bias", [P, 30], F32, kind="ExternalInput")
    out_dram = nc.dram_tensor("out", [48, H, W], F32, kind="ExternalOutput")

    planes = {
        nm: nc.alloc_sbuf_tensor(f"pl_{nm}", [P, NT * WPAD], mm_dt)
        for nm in PLANE_NAMES
    }
    ident_sb = nc.alloc_sbuf_tensor("ident_sb", [P, 5 * P], F32)
    wtab_sb = nc.alloc_sbuf_tensor("wtab_sb", [P, NSCAL], F32)
    bias_sb = nc.alloc_sbuf_tensor("bias_sb", [P, 30], F32)

    def pslice(nm, t, c0, c1):
        return planes[nm][:, t * WPAD + c0 : t * WPAD + c1]

    with tile.TileContext(nc) as tc:
        with (
            tc.tile_pool(name="io", bufs=3) as io_pool,
            tc.tile_pool(name="front", bufs=2) as fr_pool,
            tc.tile_pool(name="bands", bufs=12) as band_pool,
            tc.tile_pool(name="psum", bufs=8, space="PSUM") as psum_pool,
            tc.tile_pool(name="c4st", bufs=3) as c4_pool,
        ):
            for _it in range(loop_k):
                # ---- parameter loads ----
                for j in range(5):
                    nc.sync.dma_start(
                        out=ident_sb[:, j * P : (j + 1) * P],
                        in_=ident_dram[j * P : (j + 1) * P, :],
                    )
                nc.sync.dma_start(out=wtab_sb[:], in_=wtab_dram[:])
                nc.sync.dma_start(out=bias_sb[:], in_=bias_dram[:])

                # ---- zero W margins of all planes (written once) ----
                for nm in PLANE_NAMES:
                    for t in range(NT):
                        nc.gpsimd.memset(msafe(pslice(nm, t, 0, 2)), 0.0)
                        nc.gpsimd.memset(msafe(pslice(nm, t, 514, 516)), 0.0)

                # ---- front-end: sq ----
                # A/B garbage regions are pre-zeroed so the out-of-image rows
                # compute v=0 -> sq=0, which is exactly the reference's zero pad.
                for t in range(NT):
                    p_lo = HALO if t == 0 else 0
                    p_hi = H - (S * (NT - 1) - HALO) if t == NT - 1 else P  # 96 at t=4
                    n = p_hi - p_lo
                    r_lo = S * t - HALO + p_lo
                    A = io_pool.tile([P, W], F32, tag="A")
                    B = io_pool.tile([P, W], F32, tag="B")
                    if t == 0:
                        nc.gpsimd.memset(A[0:32, :], 0.0)
                        nc.gpsimd.memset(B[0:32, :], 0.0)
                    if t == NT - 1:
                        nc.gpsimd.memset(A[96:P, :], 0.0)
                        nc.gpsimd.memset(B[96:P, :], 0.0)
                    nc.sync.dma_start(out=A[p_lo:p_hi, :], in_=p_dram[r_lo : r_lo + n, :])
                    if t == 0:
                        nc.sync.dma_start(
                            out=B[p_lo + 1 : p_hi, :], in_=p_dram[0 : n - 1, :]
                        )
                        nc.sync.dma_start(out=B[p_lo : p_lo + 1, :], in_=p_dram[0:1, :])
                    else:
                        nc.sync.dma_start(
                            out=B[p_lo:p_hi, :], in_=p_dram[r_lo - 1 : r_lo - 1 + n, :]
                        )
                    V = fr_pool.tile([P, W], F32, tag="V")
                    K1 = fr_pool.tile([P, W], F32, tag="K1")
                    K2 = fr_pool.tile([P, W], F32, tag="K2")
                    K3 = fr_pool.tile([P, W], F32, tag="K3")
                    K4 = fr_pool.tile([P, W], F32, tag="K4")
                    ao = mybir.AluOpType
                    nc.vector.tensor_tensor(V[:], A[:], B[:], ao.subtract)
                    nc.vector.tensor_scalar(K1[:], V[:], PI, None, ao.is_ge)
                    nc.vector.tensor_scalar(K2[:], V[:], 3 * PI, None, ao.is_ge)
                    nc.vector.tensor_scalar(K3[:], V[:], -PI, None, ao.is_le)
                    nc.vector.tensor_scalar(K4[:], V[:], -3 * PI, None, ao.is_le)
                    nc.vector.tensor_tensor(K1[:], K1[:], K2[:], ao.add)
                    nc.vector.tensor_tensor(K3[:], K3[:], K4[:], ao.add)
                    nc.vector.tensor_tensor(K1[:], K1[:], K3[:], ao.subtract)
                    nc.vector.scalar_tensor_tensor(
                        V[:], K1[:], -2 * PI, V[:], ao.mult, ao.add
                    )
                    sq_dst = planes["sq"][:, t * WPAD + 2 : t * WPAD + 514]
                    nc.vector.tensor_tensor(sq_dst, V[:], V[:], ao.mult)

                # ---- convs ----
                jcol = 0
                bias_col = 0
                p_hi_last = H - (S * (NT - 1) - HALO)  # 108
                for c in range(4):
                    dil, pad_top, pad_left, KH, KW = CONV_GEOM[c]
                    in_names = CONV_INPUTS[c]
                    O = CONV_OUT[c]
                    deltas = [kh * dil - pad_top for kh in range(KH)]
                    for o in range(O):
                        psums = [
                            psum_pool.tile([P, W], F32, tag="ps", name=f"ps_{c}_{o}_{t}")
                            for t in range(NT)
                        ]
                        for ci, nm in enumerate(in_names):
                            for kw in range(KW):
                                band = band_pool.tile([P, P], mm_dt, tag="band")
                                if skip_bands:
                                    deltas_eff = []
                                    jcol += len(deltas)
                                else:
                                    deltas_eff = deltas
                                for i, d in enumerate(deltas_eff):
                                    w_ap = wtab_sb[:, jcol : jcol + 1]
                                    jcol += 1
                                    src = ident_sb[
                                        :, (d + 2) * P : (d + 3) * P
                                    ]
                                    ao = mybir.AluOpType
                                    if i == 0:
                                        nc.vector.tensor_scalar(
                                            band[:], src, w_ap, None, ao.mult
                                        )
                                    else:
                                        nc.vector.scalar_tensor_tensor(
                                            band[:], src, w_ap, band[:], ao.mult, ao.add
                                        )
                                coff = 2 + kw * dil - pad_left
                                first = ci == 0 and kw == 0
                                last = ci == len(in_names) - 1 and kw == KW - 1
                                for t in range(NT):
                                    rhs = planes[nm][
                                        :, t * WPAD + coff : t * WPAD + coff + W
                                    ]
                                    nc.tensor.matmul(
                                        psums[t],
                                        (
                                            ident_sb[:, 2 * P : 3 * P].bitcast(mm_dt)
                                            if mm != "bf16"
                                            else ident_sb[:, 2 * P : 3 * P]
                                        )
                                        if skip_bands
                                        else band[:],
                                        rhs,
                                        start=first,
                                        stop=last,
                                    )
                        bias_ap = bias_sb[:, bias_col + o : bias_col + o + 1]
                        if c < 3:
                            out_nm = (
                                ["c1_0", "c1_1"][o]
                                if c == 0
                                else (f"c2_{o}" if c == 1 else f"c3_{o}")
                            )
                            for t in range(NT):
                                nc.scalar.add(
                                    pslice(out_nm, t, 2, 514), psums[t][:], bias_ap
                                )
                        else:
                            for t in range(NT):
                                st = c4_pool.tile([P, W], F32, tag="c4")
                                nc.scalar.add(st[:], psums[t][:], bias_ap)
                                rows = S if t < NT - 1 else H - S * (NT - 1)
                                nc.sync.dma_start(
                                    out=out_dram[o, S * t : S * t + rows, :],
                                    in_=st[HALO : HALO + rows, :],
                                )
                    # edge-zero the new planes (reference 'same' zero padding)
                    if c < 3:
                        outs = (
                            ["c1_0", "c1_1"]
                            if c == 0
                            else (
                                [f"c2_{i}" for i in range(4)]
                                if c == 1
                                else [f"c3_{i}" for i in range(8)]
                            )
                        )
                        for nm in outs:
                            nc.gpsimd.memset(msafe(planes[nm][0:HALO, 0:WPAD]), 0.0)
                            nc.gpsimd.memset(
                                msafe(
                                    planes[nm][
                                        p_hi_last:P, (NT - 1) * WPAD : NT * WPAD
                                    ]
                                ),
                                0.0,
                            )
                    bias_col += O

                # ---- remaining output channels from stored planes ----
                for ch in range(16, 48 if out_mode == 'full' else 16):
                    nm = CH_MAP[ch]
                    for t in range(NT):
                        rows = S if t < NT - 1 else H - S * (NT - 1)
                        src_ap = planes[nm][
                            HALO : HALO + rows, t * WPAD + 2 : t * WPAD + 514
                        ]
                        if mm == 'bf16':
                            nc.gpsimd.dma_start(
                                out=out_dram[ch, S * t : S * t + rows, :],
                                in_=src_ap,
                            )
                        else:
                            nc.sync.dma_start(
                                out=out_dram[ch, S * t : S * t + rows, :],
                                in_=src_ap.bitcast(F32),
                            )

    nc.compile()
    return nc


_NC_CACHE = None


def _get_nc():
    global _NC_CACHE
    if _NC_CACHE is None:
        _NC_CACHE = build_nc(mm=DEFAULT_MM)
    return _NC_CACHE


def _run(inputs, trace=False):
    inputs = {k: np.asarray(v) for k, v in inputs.items()}
    nc = _get_nc()
    wtab, ident, bias = _host_tables(inputs)
    feat = inputs["feature_in"].astype(np.float32)  # [8,1,512,512]
    n_cores = feat.shape[0]
    in_maps = [
        {"p": feat[b, 0], "ident": ident, "wtab": wtab, "bias": bias}
        for b in range(n_cores)
    ]
    res = bass_utils.run_bass_kernel_spmd(
        nc, in_maps, core_ids=list(range(n_cores)), trace=trace
    )
    out = np.stack([res.results[b]["out"] for b in range(n_cores)], axis=0)
    return out.astype(np.float32), res


def kernel(**inputs):
    return _run(inputs, trace=False)[0]



# revision 3
# speedup vs baseline: 1.0606x; 1.0606x over previous
"""Trainium2 Bass kernel for nn_Encoder_P, v2.

Changes vs v1 (banded everything, f32r, 48ch f32 out):
  - S=104 row tiling (mod-8 aligned) so conv3/conv4 switch to packed-
    contraction matmuls: conv3 reads an assembled [(7ci x 11r)=77, 516]
    tile per 8-row block (2 matmuls/block, kh folded into lhsT); conv4
    reads pk4 [(8 c3ch x 8r)+(7 low x 8r)=120, 512] (1 matmul/block).
    PE work drops ~4.8x (1980 -> ~460 matmuls equivalent).
  - lhsT tables for conv3/conv4 are host-built and DMA'd (no on-device
    band construction for them).
  - All planes bf16; output is 31 deduped channels in bf16; host
    upconverts to f32 and replicates the 17 duplicate concat channels.
"""

import numpy as np
import ml_dtypes

import concourse.bacc as bacc
import concourse.bass as bass
import concourse.mybir as mybir
import concourse.tile as tile
from concourse import bass_utils

F32 = mybir.dt.float32
BF16 = mybir.dt.bfloat16
BF16_NP = ml_dtypes.bfloat16

H = 512
W = 512
S = 104          # tile stride (mod 8 == 0); last tile canonical = 96 rows
HALO = 12
NT = 5
WPAD = 516
P = 128
PI = float(np.pi)
NB = 64          # 8-row blocks
NKT = [13, 13, 13, 13, 12]   # blocks per tile

# banded conv specs (conv1, conv2): (dil, pad_top, pad_left, KH, KW)
CONV_GEOM = [(1, 1, 1, 4, 4), (2, 2, 2, 3, 3)]
PLANE_NAMES = ["sq", "c1_0", "c1_1", "c2_0", "c2_1", "c2_2", "c2_3"]
CONV_INPUTS = [["sq"], ["c1_0", "c1_1", "sq"]]
CONV_OUT = [2, 4]
DELTAS = [-2, -1, 0, 1, 2]
LOW7 = ["c2_0", "c2_1", "c2_2", "c2_3", "c1_0", "c1_1", "sq"]
NSCAL = sum(
    CONV_OUT[c] * len(CONV_INPUTS[c]) * CONV_GEOM[c][3] * CONV_GEOM[c][4]
    for c in range(2)
)  # 140

# dedup output channel order: c4 x16, c3 x8, c2 x4, c1 x2, sq
N_OUT = 31
FULL_CH = (
    [f"c4_{i}" for i in range(16)]
    + [f"c3_{i}" for i in range(8)]
    + [f"c2_{i}" for i in range(4)]
    + ["c1_0", "c1_1", "sq", "sq", "c1_0", "c1_1", "sq", "sq"]
    + [f"c2_{i}" for i in range(4)]
    + ["c1_0", "c1_1", "sq", "sq"]
    + ["c1_0", "c1_1", "sq", "sq"]
)
IDX31 = {}
for i in range(16):
    IDX31[f"c4_{i}"] = i
for i in range(8):
    IDX31[f"c3_{i}"] = 16 + i
for i in range(4):
    IDX31[f"c2_{i}"] = 24 + i
IDX31["c1_0"] = 28
IDX31["c1_1"] = 29
IDX31["sq"] = 30


def _fold_weights(w1, w2, w3, w4):
    w3f = np.zeros((8, 7, 2, 2), np.float32)
    w3f[:, :6] = w3[:, :6]
    w3f[:, 6] = w3[:, 6] + w3[:, 7]
    w4f = np.zeros((16, 15), np.float32)
    w4f[:, :12] = w4[:, :12, 0, 0]
    w4f[:, 12] = w4[:, 12, 0, 0] + w4[:, 16, 0, 0]
    w4f[:, 13] = w4[:, 13, 0, 0] + w4[:, 17, 0, 0]
    w4f[:, 14] = (
        w4[:, 14, 0, 0] + w4[:, 15, 0, 0] + w4[:, 18, 0, 0] + w4[:, 19, 0, 0]
    )
    return w1.astype(np.float32), w2.astype(np.float32), w3f, w4f


def _host_tables(inputs):
    w1, w2, w3f, w4f = _fold_weights(
        inputs["w1"], inputs["w2"], inputs["w3"], inputs["w4"]
    )
    wf = [w1, w2]
    scal = []
    for c in range(2):
        dil, pad_top, _, KH, KW = CONV_GEOM[c]
        for o in range(CONV_OUT[c]):
            for ci in range(len(CONV_INPUTS[c])):
                for kw in range(KW):
                    for kh in range(KH):
                        scal.append(wf[c][o, ci, kh, kw])
    assert len(scal) == NSCAL
    wtab = np.tile(np.asarray(scal, np.float32)[None, :], (P, 1))
    ident = np.concatenate(
        [np.eye(P, dtype=np.float32, k=-d) for d in DELTAS], axis=0
    )
    # conv1/conv2 drain biases [128, 6]
    bias12 = np.concatenate([inputs["b1"], inputs["b2"]]).astype(np.float32)
    bias12 = np.tile(bias12[None, :], (P, 1))
    # conv3 drain bias [64->128, 1], conv4 [128, 1]
    bias3 = np.zeros((P, 1), np.float32)
    bias3[:64, 0] = np.asarray(inputs["b3"], np.float32)[np.arange(64) // 8]
    bias4 = np.asarray(inputs["b4"], np.float32)[np.arange(P) // 8][:, None]
    # lhsT3 [77, 2, 64]: q=11*ci+r -> col p=8*o+ro, taps r=ro+3*kh
    lhsT3 = np.zeros((77, 2, 64), np.float32)
    for ci in range(7):
        for o in range(8):
            for ro in range(8):
                for kh in range(2):
                    for kw in range(2):
                        lhsT3[11 * ci + ro + 3 * kh, kw, 8 * o + ro] = w3f[
                            o, ci, kh, kw
                        ]
    # lhsT4 [120, 128]: q<64: c3 part; q>=64: low7 part
    lhsT4 = np.zeros((120, P), np.float32)
    for o in range(16):
        for ro in range(8):
            p = 8 * o + ro
            for ci in range(8):
                lhsT4[8 * ci + ro, p] = w4f[o, ci]
            for ci7 in range(7):
                lhsT4[64 + 8 * ci7 + ro, p] = w4f[o, 8 + ci7]
    return {
        "wtab": wtab,
        "ident": ident,
        "bias12": bias12,
        "bias3": bias3,
        "bias4": bias4,
        "lhsT3": lhsT3.astype(BF16_NP),
        "lhsT4": lhsT4.astype(BF16_NP),
    }


def _sub_ap(t_ap, dims, offset):
    """Custom (possibly overlapped) AP over a tensor's element space."""
    c = t_ap.copy()
    c.ap = type(c.ap)([list(d) for d in dims])
    c.offset = offset
    return c


def build_nc(loop_k=1):
    nc = bacc.Bacc("TRN2", target_bir_lowering=False, debug=False)
    ao = mybir.AluOpType

    p_dram = nc.dram_tensor("p", [H, W], F32, kind="ExternalInput")
    ident_dram = nc.dram_tensor("ident", [5 * P, P], F32, kind="ExternalInput")
    wtab_dram = nc.dram_tensor("wtab", [P, NSCAL], F32, kind="ExternalInput")
    bias12_dram = nc.dram_tensor("bias12", [P, 6], F32, kind="ExternalInput")
    bias3_dram = nc.dram_tensor("bias3", [P, 1], F32, kind="ExternalInput")
    bias4_dram = nc.dram_tensor("bias4", [P, 1], F32, kind="ExternalInput")
    lt3_dram = nc.dram_tensor("lhsT3", [77, 2, 64], BF16, kind="ExternalInput")
    lt4_dram = nc.dram_tensor("lhsT4", [120, P], BF16, kind="ExternalInput")
    out_dram = nc.dram_tensor("out", [N_OUT, H, W], BF16, kind="ExternalOutput")

    zeros_dram = nc.dram_tensor("zeros", [2, WPAD], BF16, kind="ExternalInput")
    planes = {
        nm: nc.alloc_sbuf_tensor(f"pl_{nm}", [P, NT, WPAD], BF16)
        for nm in PLANE_NAMES
    }
    pk4 = nc.alloc_sbuf_tensor("pk4", [P, NB, W], BF16)
    rhs3 = nc.alloc_sbuf_tensor("rhs3", [P, 3, 13, WPAD], BF16)
    ident_sb = nc.alloc_sbuf_tensor("ident_sb", [P, 5 * P], F32)
    wtab_sb = nc.alloc_sbuf_tensor("wtab_sb", [P, NSCAL], F32)
    bias12_sb = nc.alloc_sbuf_tensor("bias12_sb", [P, 6], F32)
    bias3_sb = nc.alloc_sbuf_tensor("bias3_sb", [P, 1], F32)
    bias4_sb = nc.alloc_sbuf_tensor("bias4_sb", [P, 1], F32)
    lt3_sb = nc.alloc_sbuf_tensor("lt3_sb", [77, 2, 64], BF16)
    lt4_sb = nc.alloc_sbuf_tensor("lt4_sb", [120, P], BF16)

    PLE = NT * WPAD          # plane partition stride, elements
    PK4E = NB * W            # pk4 partition stride
    R3E = 3 * 13 * WPAD      # rhs3 partition stride
    R3PAR = [0, 1, 0, 1, 2]  # rhs3 buffer slot per tile (t4 has its own)
    # out_dram channel index per LOW7 plane
    LOWCH = [IDX31[nm] for nm in LOW7]

    def plane_out(nm, t):
        """DMA canonical rows of plane tile t to its output channel."""
        ch = IDX31[nm]
        rows = 96 if t == NT - 1 else S
        src = planes[nm][HALO : HALO + rows, t, 2:514]
        dst = _sub_ap(
            out_dram.ap(), [[W, rows], [1, W]], ch * H * W + S * t * W
        )
        nc.scalar.dma_start(out=dst, in_=src)

    with tile.TileContext(nc) as tc:
        with (
            tc.tile_pool(name="io", bufs=3) as io_pool,
            tc.tile_pool(name="front", bufs=2) as fr_pool,
            tc.tile_pool(name="bands", bufs=8) as band_pool,
            tc.tile_pool(name="psum", bufs=3, space="PSUM") as psum_pool,
            tc.tile_pool(name="psum3", bufs=2, space="PSUM") as psum3_pool,
            tc.tile_pool(name="psum4", bufs=3, space="PSUM") as psum4_pool,
            tc.tile_pool(name="c4st", bufs=4) as c4_pool,
        ):
            for _it in range(loop_k):
                # ---- parameter loads ----
                for j in range(5):
                    nc.sync.dma_start(
                        out=ident_sb[:, j * P : (j + 1) * P],
                        in_=ident_dram[j * P : (j + 1) * P, :],
                    )
                nc.sync.dma_start(out=wtab_sb[:], in_=wtab_dram[:])
                nc.sync.dma_start(out=bias12_sb[:], in_=bias12_dram[:])
                nc.sync.dma_start(out=bias3_sb[:], in_=bias3_dram[:])
                nc.sync.dma_start(out=bias4_sb[:], in_=bias4_dram[:])
                nc.sync.dma_start(out=lt3_sb[:], in_=lt3_dram[:])
                nc.sync.dma_start(out=lt4_sb[:], in_=lt4_dram[:])

                # ---- one-time zeroing: W margins + bottom tails ----
                for nm in PLANE_NAMES:
                    for t in range(NT):
                        nc.gpsimd.memset(planes[nm][:, t, 0:2], 0.0)
                        nc.gpsimd.memset(planes[nm][:, t, 514:516], 0.0)
                    # tail rows of last tile (drains only write [0:108])
                    nc.gpsimd.memset(planes[nm][96:P, NT - 1, :], 0.0)

                # ---- front-end: sq = wrap(diff)^2 ----
                for t in range(NT):
                    p_lo = HALO if t == 0 else 0
                    p_hi = 108 if t == NT - 1 else P
                    n = p_hi - p_lo
                    r_lo = S * t - HALO + p_lo
                    A = io_pool.tile([P, W], F32, tag="A")
                    B = io_pool.tile([P, W], F32, tag="B")
                    if t == 0:
                        nc.gpsimd.memset(A[0:32, :], 0.0)
                        nc.gpsimd.memset(B[0:32, :], 0.0)
                    if t == NT - 1:
                        nc.gpsimd.memset(A[96:P, :], 0.0)
                        nc.gpsimd.memset(B[96:P, :], 0.0)
                    nc.sync.dma_start(
                        out=A[p_lo:p_hi, :], in_=p_dram[r_lo : r_lo + n, :]
                    )
                    if t == 0:
                        nc.sync.dma_start(
                            out=B[p_lo + 1 : p_hi, :], in_=p_dram[0 : n - 1, :]
                        )
                        nc.sync.dma_start(
                            out=B[p_lo : p_lo + 1, :], in_=p_dram[0:1, :]
                        )
                    else:
                        nc.sync.dma_start(
                            out=B[p_lo:p_hi, :],
                            in_=p_dram[r_lo - 1 : r_lo - 1 + n, :],
                        )
                    V = fr_pool.tile([P, W], F32, tag="V")
                    K1 = fr_pool.tile([P, W], F32, tag="K1")
                    K2 = fr_pool.tile([P, W], F32, tag="K2")
                    K3 = fr_pool.tile([P, W], F32, tag="K3")
                    K4 = fr_pool.tile([P, W], F32, tag="K4")
                    nc.vector.tensor_tensor(V[:], A[:], B[:], ao.subtract)
                    nc.vector.tensor_scalar(K1[:], V[:], PI, None, ao.is_ge)
                    nc.vector.tensor_scalar(K2[:], V[:], 3 * PI, None, ao.is_ge)
                    nc.vector.tensor_scalar(K3[:], V[:], -PI, None, ao.is_le)
                    nc.vector.tensor_scalar(K4[:], V[:], -3 * PI, None, ao.is_le)
                    nc.vector.tensor_tensor(K1[:], K1[:], K2[:], ao.add)
                    nc.vector.tensor_tensor(K3[:], K3[:], K4[:], ao.add)
                    nc.vector.tensor_tensor(K1[:], K1[:], K3[:], ao.subtract)
                    nc.vector.scalar_tensor_tensor(
                        V[:], K1[:], -2 * PI, V[:], ao.mult, ao.add
                    )
                    sq_dst = planes["sq"][:, t, 2:514]
                    nc.vector.tensor_tensor(sq_dst, V[:], V[:], ao.mult)
                    plane_out("sq", t)

                # ---- conv1, conv2: banded matmuls (as v1) ----
                jcol = 0
                bias_col = 0
                for c in range(2):
                    dil, pad_top, pad_left, KH, KW = CONV_GEOM[c]
                    in_names = CONV_INPUTS[c]
                    O = CONV_OUT[c]
                    deltas = [kh * dil - pad_top for kh in range(KH)]
                    for o in range(O):
                        bands = []
                        coffs = []
                        srcs = []
                        for ci, nm in enumerate(in_names):
                            for kw in range(KW):
                                band = band_pool.tile(
                                    [P, P], BF16, tag=f"band{len(bands)}"
                                )
                                for i, d in enumerate(deltas):
                                    w_ap = wtab_sb[:, jcol : jcol + 1]
                                    jcol += 1
                                    src = ident_sb[:, (d + 2) * P : (d + 3) * P]
                                    if i == 0:
                                        nc.vector.tensor_scalar(
                                            band[:], src, w_ap, None, ao.mult
                                        )
                                    else:
                                        nc.vector.scalar_tensor_tensor(
                                            band[:], src, w_ap, band[:],
                                            ao.mult, ao.add,
                                        )
                                bands.append(band)
                                coffs.append(2 + kw * dil - pad_left)
                                srcs.append(nm)
                        out_nm = (
                            ["c1_0", "c1_1"][o] if c == 0 else f"c2_{o}"
                        )
                        bias_ap = bias12_sb[:, bias_col + o : bias_col + o + 1]
                        for t in range(NT):
                            ps = psum_pool.tile([P, W], F32, tag="ps")
                            for i, band in enumerate(bands):
                                rhs = _sub_ap(
                                    planes[srcs[i]].ap(),
                                    [[PLE, P], [1, W]],
                                    t * WPAD + coffs[i],
                                )
                                nc.tensor.matmul(
                                    ps, band[:], rhs,
                                    start=(i == 0), stop=(i == len(bands) - 1),
                                )
                            p_hi = 108 if t == NT - 1 else P
                            nc.scalar.add(
                                planes[out_nm][0:p_hi, t, 2:514],
                                ps[0:p_hi, :],
                                bias_ap[0:p_hi, :],
                            )
                    # zero top halo of the new planes (rows < 0)
                    outs = ["c1_0", "c1_1"] if c == 0 else [
                        f"c2_{i}" for i in range(4)
                    ]
                    for nm in outs:
                        nc.gpsimd.memset(planes[nm][0:HALO, 0, :], 0.0)
                        for t in range(NT):
                            plane_out(nm, t)
                    bias_col += O

                # ---- rhs3 zero margins + zero edge cells (once) ----
                nc.gpsimd.memset(rhs3[0:77, :, :, 0:2], 0.0)
                nc.gpsimd.memset(rhs3[0:77, :, :, 514:516], 0.0)
                for ci in range(7):
                    # t=0, slot 0, k=0, r'=0: image row -1 -> zero
                    dst = _sub_ap(rhs3.ap(), [[R3E, 1], [1, WPAD]], 11 * ci * R3E)
                    nc.sync.dma_start(out=dst, in_=zeros_dram[0:1, :])
                    # t=4, slot 2, k=11, r' in {9,10}: rows 512,513 -> zero
                    dst = _sub_ap(
                        rhs3.ap(),
                        [[R3E, 2], [1, WPAD]],
                        (11 * ci + 9) * R3E + 2 * 13 * WPAD + 11 * WPAD,
                    )
                    nc.sync.dma_start(out=dst, in_=zeros_dram[:])

                # ---- conv3: packed contraction, 8-row blocks ----
                for t in range(NT):
                    nk = NKT[t]
                    par = R3PAR[t]
                    for ci in range(7):
                        ch = LOWCH[ci]
                        base = 11 * ci * R3E + par * 13 * WPAD
                        if t == 0:
                            # r' 1..10 for all k
                            src = _sub_ap(
                                out_dram.ap(),
                                [[W, 10], [8 * W, nk], [1, W]],
                                ch * H * W,
                            )
                            dst = _sub_ap(
                                rhs3.ap(),
                                [[R3E, 10], [WPAD, nk], [1, W]],
                                base + R3E + 2,
                            )
                            nc.scalar.dma_start(out=dst, in_=src)
                            # r'=0 (row 8k-1) for k 1..12
                            src = _sub_ap(
                                out_dram.ap(),
                                [[8 * W, 12], [1, W]],
                                ch * H * W + 7 * W,
                            )
                            dst = _sub_ap(
                                rhs3.ap(),
                                [[R3E, 1], [WPAD, 12], [1, W]],
                                base + WPAD + 2,
                            )
                            nc.scalar.dma_start(out=dst, in_=src)
                        elif t == NT - 1:
                            # k 0..10 all r'; k=11 r' 0..8
                            src = _sub_ap(
                                out_dram.ap(),
                                [[W, 11], [8 * W, 11], [1, W]],
                                ch * H * W + (S * t - 1) * W,
                            )
                            dst = _sub_ap(
                                rhs3.ap(),
                                [[R3E, 11], [WPAD, 11], [1, W]],
                                base + 2,
                            )
                            nc.scalar.dma_start(out=dst, in_=src)
                            src = _sub_ap(
                                out_dram.ap(),
                                [[W, 9], [1, W]],
                                ch * H * W + (S * t + 87) * W,
                            )
                            dst = _sub_ap(
                                rhs3.ap(),
                                [[R3E, 9], [1, W]],
                                base + 11 * WPAD + 2,
                            )
                            nc.scalar.dma_start(out=dst, in_=src)
                        else:
                            src = _sub_ap(
                                out_dram.ap(),
                                [[W, 11], [8 * W, nk], [1, W]],
                                ch * H * W + (S * t - 1) * W,
                            )
                            dst = _sub_ap(
                                rhs3.ap(),
                                [[R3E, 11], [WPAD, nk], [1, W]],
                                base + 2,
                            )
                            nc.scalar.dma_start(out=dst, in_=src)
                    for k in range(nk):
                        b = 13 * t + k
                        ps3 = psum3_pool.tile([64, W], F32, tag="ps3")
                        for kw in range(2):
                            rhs = _sub_ap(
                                rhs3.ap(),
                                [[R3E, 77], [1, W]],
                                par * 13 * WPAD + k * WPAD + 1 + 3 * kw,
                            )
                            nc.tensor.matmul(
                                ps3,
                                lt3_sb[:, kw, :],
                                rhs,
                                start=(kw == 0),
                                stop=(kw == 1),
                            )
                        nc.scalar.add(
                            pk4[0:64, b, :], ps3[0:64, :], bias3_sb[0:64, :]
                        )

                # ---- conv4: low7 part of pk4 from output channels ----
                for ci7 in range(7):
                    ch = LOWCH[ci7]
                    src = _sub_ap(
                        out_dram.ap(),
                        [[W, 8], [8 * W, NB], [1, W]],
                        ch * H * W,
                    )
                    nc.sync.dma_start(
                        out=pk4[64 + 8 * ci7 : 72 + 8 * ci7, :, :], in_=src
                    )
                for b in range(NB):
                    ps4 = psum4_pool.tile([P, W], F32, tag="ps4")
                    nc.tensor.matmul(
                        ps4, lt4_sb[:], pk4[0:120, b, :], start=True, stop=True
                    )
                    st = c4_pool.tile([P, W], BF16, tag="c4")
                    nc.scalar.add(st[:], ps4[:], bias4_sb[:])
                    eng = [nc.sync, nc.scalar][b % 2]
                    dst = _sub_ap(
                        out_dram.ap(),
                        [[H * W, 16], [W, 8], [1, W]],
                        8 * b * W,
                    )
                    eng.dma_start(out=dst, in_=st[:])

                # ---- DMA out: c3 channels from pk4 ----
                for o in range(8):
                    ch = 16 + o
                    dst = _sub_ap(
                        out_dram.ap(),
                        [[W, 8], [8 * W, NB], [1, W]],
                        ch * H * W,
                    )
                    nc.sync.dma_start(out=dst, in_=pk4[8 * o : 8 * o + 8, :, :])

    nc.compile()
    return nc


_NC_CACHE = None


def _get_nc():
    global _NC_CACHE
    if _NC_CACHE is None:
        _NC_CACHE = build_nc()
    return _NC_CACHE


def _expand(out31):
    """[31,H,W] bf16/f32 -> [48,H,W] f32 with duplicated concat channels."""
    out31 = np.asarray(out31).astype(np.float32)
    full = np.empty((48, H, W), np.float32)
    for ch, nm in enumerate(FULL_CH):
        full[ch] = out31[IDX31[nm]]
    return full


def _run(inputs, trace=False):
    inputs = {k: np.asarray(v) for k, v in inputs.items()}
    nc = _get_nc()
    tabs = _host_tables(inputs)
    feat = inputs["feature_in"].astype(np.float32)
    n_cores = feat.shape[0]
    in_maps = [
        {
            "p": feat[b, 0],
            "ident": tabs["ident"],
            "wtab": tabs["wtab"],
            "bias12": tabs["bias12"],
            "bias3": tabs["bias3"],
            "bias4": tabs["bias4"],
            "lhsT3": tabs["lhsT3"],
            "lhsT4": tabs["lhsT4"],
            "zeros": np.zeros((2, WPAD), BF16_NP),
        }
        for b in range(n_cores)
    ]
    res = bass_utils.run_bass_kernel_spmd(
        nc, in_maps, core_ids=list(range(n_cores)), trace=trace
    )
    out = np.stack(
        [_expand(res.results[b]["out"]) for b in range(n_cores)], axis=0
    )
    return out, res


def kernel(**inputs):
    return _run(inputs, trace=False)[0]


# revision 10
# speedup vs baseline: 1.0746x; 1.0132x over previous
"""Trainium2 Bass kernel for nn_Encoder_P, v2.

Changes vs v1 (banded everything, f32r, 48ch f32 out):
  - S=104 row tiling (mod-8 aligned) so conv3/conv4 switch to packed-
    contraction matmuls: conv3 reads an assembled [(7ci x 11r)=77, 516]
    tile per 8-row block (2 matmuls/block, kh folded into lhsT); conv4
    reads pk4 [(8 c3ch x 8r)+(7 low x 8r)=120, 512] (1 matmul/block).
    PE work drops ~4.8x (1980 -> ~460 matmuls equivalent).
  - lhsT tables for conv3/conv4 are host-built and DMA'd (no on-device
    band construction for them).
  - All planes bf16; output is 31 deduped channels in bf16; host
    upconverts to f32 and replicates the 17 duplicate concat channels.
"""

import numpy as np
import ml_dtypes

import concourse.bacc as bacc
import concourse.bass as bass
import concourse.mybir as mybir
import concourse.tile as tile
from concourse import bass_utils

F32 = mybir.dt.float32
BF16 = mybir.dt.bfloat16
BF16_NP = ml_dtypes.bfloat16

H = 512
W = 512
S = 104          # tile stride (mod 8 == 0); last tile canonical = 96 rows
HALO = 12
NT = 5
WPAD = 516
P = 128
PI = float(np.pi)
NB = 64          # 8-row blocks
NKT = [13, 13, 13, 13, 12]   # blocks per tile

# banded conv specs (conv1, conv2): (dil, pad_top, pad_left, KH, KW)
CONV_GEOM = [(1, 1, 1, 4, 4), (2, 2, 2, 3, 3)]
PLANE_NAMES = ["sq", "c1_0", "c1_1", "c2_0", "c2_1", "c2_2", "c2_3"]
CONV_INPUTS = [["sq"], ["c1_0", "c1_1", "sq"]]
CONV_OUT = [2, 4]
DELTAS = [-2, -1, 0, 1, 2]
LOW7 = ["c2_0", "c2_1", "c2_2", "c2_3", "c1_0", "c1_1", "sq"]
NSCAL = sum(
    CONV_OUT[c] * len(CONV_INPUTS[c]) * CONV_GEOM[c][3] * CONV_GEOM[c][4]
    for c in range(2)
)  # 140

# dedup output channel order: c4 x16, c3 x8, c2 x4, c1 x2, sq
N_OUT = 31
FULL_CH = (
    [f"c4_{i}" for i in range(16)]
    + [f"c3_{i}" for i in range(8)]
    + [f"c2_{i}" for i in range(4)]
    + ["c1_0", "c1_1", "sq", "sq", "c1_0", "c1_1", "sq", "sq"]
    + [f"c2_{i}" for i in range(4)]
    + ["c1_0", "c1_1", "sq", "sq"]
    + ["c1_0", "c1_1", "sq", "sq"]
)
IDX31 = {}
for i in range(16):
    IDX31[f"c4_{i}"] = i
for i in range(8):
    IDX31[f"c3_{i}"] = 16 + i
for i in range(4):
    IDX31[f"c2_{i}"] = 24 + i
IDX31["c1_0"] = 28
IDX31["c1_1"] = 29
IDX31["sq"] = 30


def _fold_weights(w1, w2, w3, w4):
    w3f = np.zeros((8, 7, 2, 2), np.float32)
    w3f[:, :6] = w3[:, :6]
    w3f[:, 6] = w3[:, 6] + w3[:, 7]
    w4f = np.zeros((16, 15), np.float32)
    w4f[:, :12] = w4[:, :12, 0, 0]
    w4f[:, 12] = w4[:, 12, 0, 0] + w4[:, 16, 0, 0]
    w4f[:, 13] = w4[:, 13, 0, 0] + w4[:, 17, 0, 0]
    w4f[:, 14] = (
        w4[:, 14, 0, 0] + w4[:, 15, 0, 0] + w4[:, 18, 0, 0] + w4[:, 19, 0, 0]
    )
    return w1.astype(np.float32), w2.astype(np.float32), w3f, w4f


def _host_tables(inputs):
    w1, w2, w3f, w4f = _fold_weights(
        inputs["w1"], inputs["w2"], inputs["w3"], inputs["w4"]
    )
    wf = [w1, w2]
    scal = []
    for c in range(2):
        dil, pad_top, _, KH, KW = CONV_GEOM[c]
        for o in range(CONV_OUT[c]):
            for ci in range(len(CONV_INPUTS[c])):
                for kw in range(KW):
                    for kh in range(KH):
                        scal.append(wf[c][o, ci, kh, kw])
    assert len(scal) == NSCAL
    # host-built banded lhsT for conv1/conv2: one [128,128] per (c,o,ci,kw)
    bands = []
    for c in range(2):
        dil, pad_top, _, KH, KW = CONV_GEOM[c]
        for o in range(CONV_OUT[c]):
            for ci in range(len(CONV_INPUTS[c])):
                for kw in range(KW):
                    band = np.zeros((P, P), np.float32)
                    for kh in range(KH):
                        d = kh * dil - pad_top
                        band += wf[c][o, ci, kh, kw] * np.eye(
                            P, dtype=np.float32, k=-d
                        )
                    bands.append(band)
    lhsT12 = np.concatenate(bands, axis=1)  # [128, 44*128]
    # conv1/conv2 drain biases [128, 6]
    bias12 = np.concatenate([inputs["b1"], inputs["b2"]]).astype(np.float32)
    bias12 = np.tile(bias12[None, :], (P, 1))
    # conv3 drain bias [64->128, 1], conv4 [128, 1]
    bias3 = np.zeros((P, 1), np.float32)
    bias3[:64, 0] = np.asarray(inputs["b3"], np.float32)[np.arange(64) // 8]
    bias4 = np.asarray(inputs["b4"], np.float32)[np.arange(P) // 8][:, None]
    # lhsT3 [77, 2, 64]: q=11*ci+r -> col p=8*o+ro, taps r=ro+3*kh
    lhsT3 = np.zeros((77, 2, 64), np.float32)
    for ci in range(7):
        for o in range(8):
            for ro in range(8):
                for kh in range(2):
                    for kw in range(2):
                        lhsT3[11 * ci + ro + 3 * kh, kw, 8 * o + ro] = w3f[
                            o, ci, kh, kw
                        ]
    # lhsT4 [120, 128]: q<64: c3 part; q>=64: low7 part
    lhsT4 = np.zeros((120, P), np.float32)
    for o in range(16):
        for ro in range(8):
            p = 8 * o + ro
            for ci in range(8):
                lhsT4[8 * ci + ro, p] = w4f[o, ci]
            for ci7 in range(7):
                lhsT4[64 + 8 * ci7 + ro, p] = w4f[o, 8 + ci7]
    return {
        "lhsT12": lhsT12.astype(BF16_NP),
        "bias12": bias12,
        "bias3": bias3,
        "bias4": bias4,
        "lhsT3": lhsT3.astype(BF16_NP),
        "lhsT4": lhsT4.astype(BF16_NP),
    }


def _sub_ap(t_ap, dims, offset):
    """Custom (possibly overlapped) AP over a tensor's element space."""
    c = t_ap.copy()
    c.ap = type(c.ap)([list(d) for d in dims])
    c.offset = offset
    return c


def build_nc(loop_k=1):
    nc = bacc.Bacc("TRN2", target_bir_lowering=False, debug=False)
    ao = mybir.AluOpType

    p_dram = nc.dram_tensor("p", [H, W], F32, kind="ExternalInput")
    lt12_dram = nc.dram_tensor("lhsT12", [P, 44 * P], BF16, kind="ExternalInput")
    bias12_dram = nc.dram_tensor("bias12", [P, 6], F32, kind="ExternalInput")
    bias3_dram = nc.dram_tensor("bias3", [P, 1], F32, kind="ExternalInput")
    bias4_dram = nc.dram_tensor("bias4", [P, 1], F32, kind="ExternalInput")
    lt3_dram = nc.dram_tensor("lhsT3", [77, 2, 64], BF16, kind="ExternalInput")
    lt4_dram = nc.dram_tensor("lhsT4", [120, P], BF16, kind="ExternalInput")
    out_dram = nc.dram_tensor("out", [N_OUT, H, W], BF16, kind="ExternalOutput")

    zeros_dram = nc.dram_tensor("zeros", [2, WPAD], BF16, kind="ExternalInput")
    planes = {
        nm: nc.alloc_sbuf_tensor(f"pl_{nm}", [P, NT, WPAD], BF16)
        for nm in PLANE_NAMES
    }
    pk4 = nc.alloc_sbuf_tensor("pk4", [P, NB, W], BF16)
    rhs3 = nc.alloc_sbuf_tensor("rhs3", [P, 3, 13, WPAD], BF16)
    lt12_sb = nc.alloc_sbuf_tensor("lt12_sb", [P, 44 * P], BF16)
    bias12_sb = nc.alloc_sbuf_tensor("bias12_sb", [P, 6], F32)
    bias3_sb = nc.alloc_sbuf_tensor("bias3_sb", [P, 1], F32)
    bias4_sb = nc.alloc_sbuf_tensor("bias4_sb", [P, 1], F32)
    lt3_sb = nc.alloc_sbuf_tensor("lt3_sb", [77, 2, 64], BF16)
    lt4_sb = nc.alloc_sbuf_tensor("lt4_sb", [120, P], BF16)

    PLE = NT * WPAD          # plane partition stride, elements
    PK4E = NB * W            # pk4 partition stride
    R3E = 3 * 13 * WPAD      # rhs3 partition stride
    R3PAR = [0, 1, 0, 1, 2]  # rhs3 buffer slot per tile (t4 has its own)
    # out_dram channel index per LOW7 plane
    LOWCH = [IDX31[nm] for nm in LOW7]

    def plane_out(nm, t):
        """DMA canonical rows of plane tile t to its output channel."""
        ch = IDX31[nm]
        rows = 96 if t == NT - 1 else S
        src = planes[nm][HALO : HALO + rows, t, 2:514]
        dst = _sub_ap(
            out_dram.ap(), [[W, rows], [1, W]], ch * H * W + S * t * W
        )
        nc.scalar.dma_start(out=dst, in_=src)

    with tile.TileContext(nc) as tc:
        with (
            tc.tile_pool(name="io", bufs=3) as io_pool,
            tc.tile_pool(name="front", bufs=2) as fr_pool,
            tc.tile_pool(name="psum", bufs=3, space="PSUM") as psum_pool,
            tc.tile_pool(name="psum3", bufs=2, space="PSUM") as psum3_pool,
            tc.tile_pool(name="psum4", bufs=3, space="PSUM") as psum4_pool,
            tc.tile_pool(name="c4st", bufs=4) as c4_pool,
        ):
            for _it in range(loop_k):
                # ---- parameter loads ----
                nc.sync.dma_start(out=lt12_sb[:], in_=lt12_dram[:])
                nc.sync.dma_start(out=bias12_sb[:], in_=bias12_dram[:])
                nc.sync.dma_start(out=bias3_sb[:], in_=bias3_dram[:])
                nc.sync.dma_start(out=bias4_sb[:], in_=bias4_dram[:])
                nc.sync.dma_start(out=lt3_sb[:], in_=lt3_dram[:])
                nc.sync.dma_start(out=lt4_sb[:], in_=lt4_dram[:])

                # ---- one-time zeroing: W margins + bottom tails ----
                for nm in PLANE_NAMES:
                    for t in range(NT):
                        nc.gpsimd.memset(planes[nm][:, t, 0:2], 0.0)
                        nc.gpsimd.memset(planes[nm][:, t, 514:516], 0.0)
                    # tail rows of last tile (drains only write [0:108])
                    nc.gpsimd.memset(planes[nm][96:P, NT - 1, :], 0.0)

                # ---- front-end: sq = wrap(diff)^2 ----
                for t in range(NT):
                    p_lo = HALO if t == 0 else 0
                    p_hi = 108 if t == NT - 1 else P
                    n = p_hi - p_lo
                    r_lo = S * t - HALO + p_lo
                    A = io_pool.tile([P, W], F32, tag="A")
                    B = io_pool.tile([P, W], F32, tag="B")
                    if t == 0:
                        nc.gpsimd.memset(A[0:32, :], 0.0)
                        nc.gpsimd.memset(B[0:32, :], 0.0)
                    if t == NT - 1:
                        nc.gpsimd.memset(A[96:P, :], 0.0)
                        nc.gpsimd.memset(B[96:P, :], 0.0)
                    nc.sync.dma_start(
                        out=A[p_lo:p_hi, :], in_=p_dram[r_lo : r_lo + n, :]
                    )
                    if t == 0:
                        nc.sync.dma_start(
                            out=B[p_lo + 1 : p_hi, :], in_=p_dram[0 : n - 1, :]
                        )
                        nc.sync.dma_start(
                            out=B[p_lo : p_lo + 1, :], in_=p_dram[0:1, :]
                        )
                    else:
                        nc.sync.dma_start(
                            out=B[p_lo:p_hi, :],
                            in_=p_dram[r_lo - 1 : r_lo - 1 + n, :],
                        )
                    V = fr_pool.tile([P, W], F32, tag="V")
                    K1 = fr_pool.tile([P, W], F32, tag="K1")
                    K2 = fr_pool.tile([P, W], F32, tag="K2")
                    K3 = fr_pool.tile([P, W], F32, tag="K3")
                    K4 = fr_pool.tile([P, W], F32, tag="K4")
                    nc.vector.tensor_tensor(V[:], A[:], B[:], ao.subtract)
                    nc.vector.tensor_scalar(K1[:], V[:], PI, None, ao.is_ge)
                    nc.vector.tensor_scalar(K2[:], V[:], 3 * PI, None, ao.is_ge)
                    nc.vector.tensor_scalar(K3[:], V[:], -PI, None, ao.is_le)
                    nc.vector.tensor_scalar(K4[:], V[:], -3 * PI, None, ao.is_le)
                    nc.vector.tensor_tensor(K1[:], K1[:], K2[:], ao.add)
                    nc.vector.tensor_tensor(K3[:], K3[:], K4[:], ao.add)
                    nc.vector.tensor_tensor(K1[:], K1[:], K3[:], ao.subtract)
                    nc.vector.scalar_tensor_tensor(
                        V[:], K1[:], -2 * PI, V[:], ao.mult, ao.add
                    )
                    sq_dst = planes["sq"][:, t, 2:514]
                    nc.vector.tensor_tensor(sq_dst, V[:], V[:], ao.mult)
                    plane_out("sq", t)

                # ---- conv1, conv2: banded matmuls, host-built lhsT ----
                jband = 0
                bias_col = 0
                for c in range(2):
                    dil, pad_top, pad_left, KH, KW = CONV_GEOM[c]
                    in_names = CONV_INPUTS[c]
                    O = CONV_OUT[c]
                    for o in range(O):
                        bands = []
                        coffs = []
                        srcs = []
                        for ci, nm in enumerate(in_names):
                            for kw in range(KW):
                                bands.append(
                                    lt12_sb[:, jband * P : (jband + 1) * P]
                                )
                                jband += 1
                                coffs.append(2 + kw * dil - pad_left)
                                srcs.append(nm)
                        out_nm = (
                            ["c1_0", "c1_1"][o] if c == 0 else f"c2_{o}"
                        )
                        bias_ap = bias12_sb[:, bias_col + o : bias_col + o + 1]
                        for t in range(NT):
                            ps = psum_pool.tile([P, W], F32, tag="ps")
                            for i, band in enumerate(bands):
                                rhs = _sub_ap(
                                    planes[srcs[i]].ap(),
                                    [[PLE, P], [1, W]],
                                    t * WPAD + coffs[i],
                                )
                                nc.tensor.matmul(
                                    ps, band, rhs,
                                    start=(i == 0), stop=(i == len(bands) - 1),
                                )
                            p_hi = 108 if t == NT - 1 else P
                            nc.scalar.add(
                                planes[out_nm][0:p_hi, t, 2:514],
                                ps[0:p_hi, :],
                                bias_ap[0:p_hi, :],
                            )
                    # zero top halo of the new planes (rows < 0)
                    outs = ["c1_0", "c1_1"] if c == 0 else [
                        f"c2_{i}" for i in range(4)
                    ]
                    for nm in outs:
                        nc.gpsimd.memset(planes[nm][0:HALO, 0, :], 0.0)
                        for t in range(NT):
                            plane_out(nm, t)
                    bias_col += O

                # ---- rhs3 zero margins + zero edge cells (once) ----
                nc.gpsimd.memset(rhs3[0:77, :, :, 0:2], 0.0)
                nc.gpsimd.memset(rhs3[0:77, :, :, 514:516], 0.0)
                for ci in range(7):
                    # t=0, slot 0, k=0, r'=0: image row -1 -> zero
                    dst = _sub_ap(rhs3.ap(), [[R3E, 1], [1, WPAD]], 11 * ci * R3E)
                    nc.sync.dma_start(out=dst, in_=zeros_dram[0:1, :])
                    # t=4, slot 2, k=11, r' in {9,10}: rows 512,513 -> zero
                    dst = _sub_ap(
                        rhs3.ap(),
                        [[R3E, 2], [1, WPAD]],
                        (11 * ci + 9) * R3E + 2 * 13 * WPAD + 11 * WPAD,
                    )
                    nc.sync.dma_start(out=dst, in_=zeros_dram[:])

                # ---- conv3: packed contraction, 8-row blocks ----
                for t in range(NT):
                    nk = NKT[t]
                    par = R3PAR[t]
                    for ci in range(7):
                        ch = LOWCH[ci]
                        base = 11 * ci * R3E + par * 13 * WPAD
                        if t == 0:
                            # r' 1..10 for all k
                            src = _sub_ap(
                                out_dram.ap(),
                                [[W, 10], [8 * W, nk], [1, W]],
                                ch * H * W,
                            )
                            dst = _sub_ap(
                                rhs3.ap(),
                                [[R3E, 10], [WPAD, nk], [1, W]],
                                base + R3E + 2,
                            )
                            nc.scalar.dma_start(out=dst, in_=src)
                            # r'=0 (row 8k-1) for k 1..12
                            src = _sub_ap(
                                out_dram.ap(),
                                [[8 * W, 12], [1, W]],
                                ch * H * W + 7 * W,
                            )
                            dst = _sub_ap(
                                rhs3.ap(),
                                [[R3E, 1], [WPAD, 12], [1, W]],
                                base + WPAD + 2,
                            )
                            nc.scalar.dma_start(out=dst, in_=src)
                        elif t == NT - 1:
                            # k 0..10 all r'; k=11 r' 0..8
                            src = _sub_ap(
                                out_dram.ap(),
                                [[W, 11], [8 * W, 11], [1, W]],
                                ch * H * W + (S * t - 1) * W,
                            )
                            dst = _sub_ap(
                                rhs3.ap(),
                                [[R3E, 11], [WPAD, 11], [1, W]],
                                base + 2,
                            )
                            nc.scalar.dma_start(out=dst, in_=src)
                            src = _sub_ap(
                                out_dram.ap(),
                                [[W, 9], [1, W]],
                                ch * H * W + (S * t + 87) * W,
                            )
                            dst = _sub_ap(
                                rhs3.ap(),
                                [[R3E, 9], [1, W]],
                                base + 11 * WPAD + 2,
                            )
                            nc.scalar.dma_start(out=dst, in_=src)
                        else:
                            src = _sub_ap(
                                out_dram.ap(),
                                [[W, 11], [8 * W, nk], [1, W]],
                                ch * H * W + (S * t - 1) * W,
                            )
                            dst = _sub_ap(
                                rhs3.ap(),
                                [[R3E, 11], [WPAD, nk], [1, W]],
                                base + 2,
                            )
                            nc.scalar.dma_start(out=dst, in_=src)
                    for k in range(nk):
                        b = 13 * t + k
                        ps3 = psum3_pool.tile([64, W], F32, tag="ps3")
                        for kw in range(2):
                            rhs = _sub_ap(
                                rhs3.ap(),
                                [[R3E, 77], [1, W]],
                                par * 13 * WPAD + k * WPAD + 1 + 3 * kw,
                            )
                            nc.tensor.matmul(
                                ps3,
                                lt3_sb[:, kw, :],
                                rhs,
                                start=(kw == 0),
                                stop=(kw == 1),
                            )
                        nc.vector.tensor_scalar(
                            pk4[0:64, b, :], ps3[0:64, :],
                            bias3_sb[0:64, :], None, ao.add,
                        )

                # ---- conv4: low7 part of pk4 from output channels ----
                for ci7 in range(7):
                    ch = LOWCH[ci7]
                    src = _sub_ap(
                        out_dram.ap(),
                        [[W, 8], [8 * W, NB], [1, W]],
                        ch * H * W,
                    )
                    nc.sync.dma_start(
                        out=pk4[64 + 8 * ci7 : 72 + 8 * ci7, :, :], in_=src
                    )
                for b in range(NB):
                    ps4 = psum4_pool.tile([P, W], F32, tag="ps4")
                    nc.tensor.matmul(
                        ps4, lt4_sb[:], pk4[0:120, b, :], start=True, stop=True
                    )
                    st = c4_pool.tile([P, W], BF16, tag="c4")
                    nc.scalar.add(st[:], ps4[:], bias4_sb[:])
                    eng = [nc.sync, nc.scalar][b % 2]
                    dst = _sub_ap(
                        out_dram.ap(),
                        [[H * W, 16], [W, 8], [1, W]],
                        8 * b * W,
                    )
                    eng.dma_start(out=dst, in_=st[:])

                # ---- DMA out: c3 channels from pk4 ----
                for o in range(8):
                    ch = 16 + o
                    dst = _sub_ap(
                        out_dram.ap(),
                        [[W, 8], [8 * W, NB], [1, W]],
                        ch * H * W,
                    )
                    nc.sync.dma_start(out=dst, in_=pk4[8 * o : 8 * o + 8, :, :])

    nc.compile()
    return nc


_NC_CACHE = None


def _get_nc():
    global _NC_CACHE
    if _NC_CACHE is None:
        _NC_CACHE = build_nc()
    return _NC_CACHE


def _expand(out31):
    """[31,H,W] bf16/f32 -> [48,H,W] f32 with duplicated concat channels."""
    out31 = np.asarray(out31).astype(np.float32)
    full = np.empty((48, H, W), np.float32)
    for ch, nm in enumerate(FULL_CH):
        full[ch] = out31[IDX31[nm]]
    return full


def _run(inputs, trace=False):
    inputs = {k: np.asarray(v) for k, v in inputs.items()}
    nc = _get_nc()
    tabs = _host_tables(inputs)
    feat = inputs["feature_in"].astype(np.float32)
    n_cores = feat.shape[0]
    in_maps = [
        {
            "p": feat[b, 0],
            "lhsT12": tabs["lhsT12"],
            "bias12": tabs["bias12"],
            "bias3": tabs["bias3"],
            "bias4": tabs["bias4"],
            "lhsT3": tabs["lhsT3"],
            "lhsT4": tabs["lhsT4"],
            "zeros": np.zeros((2, WPAD), BF16_NP),
        }
        for b in range(n_cores)
    ]
    res = bass_utils.run_bass_kernel_spmd(
        nc, in_maps, core_ids=list(range(n_cores)), trace=trace
    )
    out = np.stack(
        [_expand(res.results[b]["out"]) for b in range(n_cores)], axis=0
    )
    return out, res


def kernel(**inputs):
    return _run(inputs, trace=False)[0]


# revision 11
# speedup vs baseline: 1.0938x; 1.0179x over previous
"""Trainium2 Bass kernel for nn_Encoder_P, v2.

Changes vs v1 (banded everything, f32r, 48ch f32 out):
  - S=104 row tiling (mod-8 aligned) so conv3/conv4 switch to packed-
    contraction matmuls: conv3 reads an assembled [(7ci x 11r)=77, 516]
    tile per 8-row block (2 matmuls/block, kh folded into lhsT); conv4
    reads pk4 [(8 c3ch x 8r)+(7 low x 8r)=120, 512] (1 matmul/block).
    PE work drops ~4.8x (1980 -> ~460 matmuls equivalent).
  - lhsT tables for conv3/conv4 are host-built and DMA'd (no on-device
    band construction for them).
  - All planes bf16; output is 31 deduped channels in bf16; host
    upconverts to f32 and replicates the 17 duplicate concat channels.
"""

import numpy as np
import ml_dtypes

import concourse.bacc as bacc
import concourse.bass as bass
import concourse.mybir as mybir
import concourse.tile as tile
from concourse import bass_utils

F32 = mybir.dt.float32
BF16 = mybir.dt.bfloat16
BF16_NP = ml_dtypes.bfloat16

H = 512
W = 512
S = 104          # tile stride (mod 8 == 0); last tile canonical = 96 rows
HALO = 12
NT = 5
WPAD = 516
P = 128
PI = float(np.pi)
NB = 64          # 8-row blocks
NKT = [13, 13, 13, 13, 12]   # blocks per tile

# banded conv specs (conv1, conv2): (dil, pad_top, pad_left, KH, KW)
CONV_GEOM = [(1, 1, 1, 4, 4), (2, 2, 2, 3, 3)]
PLANE_NAMES = ["sq", "c1_0", "c1_1", "c2_0", "c2_1", "c2_2", "c2_3"]
CONV_INPUTS = [["sq"], ["c1_0", "c1_1", "sq"]]
CONV_OUT = [2, 4]
DELTAS = [-2, -1, 0, 1, 2]
LOW7 = ["c2_0", "c2_1", "c2_2", "c2_3", "c1_0", "c1_1", "sq"]
NSCAL = sum(
    CONV_OUT[c] * len(CONV_INPUTS[c]) * CONV_GEOM[c][3] * CONV_GEOM[c][4]
    for c in range(2)
)  # 140

# dedup output channel order: c4 x16, c3 x8, c2 x4, c1 x2, sq
N_OUT = 31
FULL_CH = (
    [f"c4_{i}" for i in range(16)]
    + [f"c3_{i}" for i in range(8)]
    + [f"c2_{i}" for i in range(4)]
    + ["c1_0", "c1_1", "sq", "sq", "c1_0", "c1_1", "sq", "sq"]
    + [f"c2_{i}" for i in range(4)]
    + ["c1_0", "c1_1", "sq", "sq"]
    + ["c1_0", "c1_1", "sq", "sq"]
)
IDX31 = {}
for i in range(16):
    IDX31[f"c4_{i}"] = i
for i in range(8):
    IDX31[f"c3_{i}"] = 16 + i
for i in range(4):
    IDX31[f"c2_{i}"] = 24 + i
IDX31["c1_0"] = 28
IDX31["c1_1"] = 29
IDX31["sq"] = 30


def _fold_weights(w1, w2, w3, w4):
    w3f = np.zeros((8, 7, 2, 2), np.float32)
    w3f[:, :6] = w3[:, :6]
    w3f[:, 6] = w3[:, 6] + w3[:, 7]
    w4f = np.zeros((16, 15), np.float32)
    w4f[:, :12] = w4[:, :12, 0, 0]
    w4f[:, 12] = w4[:, 12, 0, 0] + w4[:, 16, 0, 0]
    w4f[:, 13] = w4[:, 13, 0, 0] + w4[:, 17, 0, 0]
    w4f[:, 14] = (
        w4[:, 14, 0, 0] + w4[:, 15, 0, 0] + w4[:, 18, 0, 0] + w4[:, 19, 0, 0]
    )
    return w1.astype(np.float32), w2.astype(np.float32), w3f, w4f


def _host_tables(inputs):
    w1, w2, w3f, w4f = _fold_weights(
        inputs["w1"], inputs["w2"], inputs["w3"], inputs["w4"]
    )
    wf = [w1, w2]
    scal = []
    for c in range(2):
        dil, pad_top, _, KH, KW = CONV_GEOM[c]
        for o in range(CONV_OUT[c]):
            for ci in range(len(CONV_INPUTS[c])):
                for kw in range(KW):
                    for kh in range(KH):
                        scal.append(wf[c][o, ci, kh, kw])
    assert len(scal) == NSCAL
    # host-built banded lhsT for conv1/conv2: one [128,128] per (c,o,ci,kw)
    bands = []
    for c in range(2):
        dil, pad_top, _, KH, KW = CONV_GEOM[c]
        for o in range(CONV_OUT[c]):
            for ci in range(len(CONV_INPUTS[c])):
                for kw in range(KW):
                    band = np.zeros((P, P), np.float32)
                    for kh in range(KH):
                        d = kh * dil - pad_top
                        band += wf[c][o, ci, kh, kw] * np.eye(
                            P, dtype=np.float32, k=-d
                        )
                    bands.append(band)
    lhsT12 = np.concatenate(bands, axis=1)  # [128, 44*128]
    # conv1/conv2 drain biases [128, 6]
    bias12 = np.concatenate([inputs["b1"], inputs["b2"]]).astype(np.float32)
    bias12 = np.tile(bias12[None, :], (P, 1))
    # conv3 drain bias [64->128, 1], conv4 [128, 1]
    bias3 = np.zeros((P, 1), np.float32)
    bias3[:64, 0] = np.asarray(inputs["b3"], np.float32)[np.arange(64) // 8]
    bias4 = np.asarray(inputs["b4"], np.float32)[np.arange(P) // 8][:, None]
    # lhsT3 [77, 2, 64]: q=11*ci+r -> col p=8*o+ro, taps r=ro+3*kh
    lhsT3 = np.zeros((77, 2, 64), np.float32)
    for ci in range(7):
        for o in range(8):
            for ro in range(8):
                for kh in range(2):
                    for kw in range(2):
                        lhsT3[11 * ci + ro + 3 * kh, kw, 8 * o + ro] = w3f[
                            o, ci, kh, kw
                        ]
    # lhsT4 [120, 128]: q<64: c3 part; q>=64: low7 part
    lhsT4 = np.zeros((120, P), np.float32)
    for o in range(16):
        for ro in range(8):
            p = 8 * o + ro
            for ci in range(8):
                lhsT4[8 * ci + ro, p] = w4f[o, ci]
            for ci7 in range(7):
                lhsT4[64 + 8 * ci7 + ro, p] = w4f[o, 8 + ci7]
    return {
        "lhsT12": lhsT12.astype(BF16_NP),
        "bias12": bias12,
        "bias3": bias3,
        "bias4": bias4,
        "lhsT3": lhsT3.astype(BF16_NP),
        "lhsT4": lhsT4.astype(BF16_NP),
    }


def _sub_ap(t_ap, dims, offset):
    """Custom (possibly overlapped) AP over a tensor's element space."""
    c = t_ap.copy()
    c.ap = type(c.ap)([list(d) for d in dims])
    c.offset = offset
    return c


def build_nc(loop_k=1):
    nc = bacc.Bacc("TRN2", target_bir_lowering=False, debug=False)
    ao = mybir.AluOpType

    p_dram = nc.dram_tensor("p", [H, W], F32, kind="ExternalInput")
    lt12_dram = nc.dram_tensor("lhsT12", [P, 44 * P], BF16, kind="ExternalInput")
    bias12_dram = nc.dram_tensor("bias12", [P, 6], F32, kind="ExternalInput")
    bias3_dram = nc.dram_tensor("bias3", [P, 1], F32, kind="ExternalInput")
    bias4_dram = nc.dram_tensor("bias4", [P, 1], F32, kind="ExternalInput")
    lt3_dram = nc.dram_tensor("lhsT3", [77, 2, 64], BF16, kind="ExternalInput")
    lt4_dram = nc.dram_tensor("lhsT4", [120, P], BF16, kind="ExternalInput")
    out_dram = nc.dram_tensor("out", [N_OUT, H, W], BF16, kind="ExternalOutput")

    zeros_dram = nc.dram_tensor("zeros", [2, WPAD], BF16, kind="ExternalInput")
    planes = {
        nm: nc.alloc_sbuf_tensor(f"pl_{nm}", [P, NT, WPAD], BF16)
        for nm in PLANE_NAMES
    }
    pk4 = nc.alloc_sbuf_tensor("pk4", [P, NB, W], BF16)
    rhs3 = nc.alloc_sbuf_tensor("rhs3", [P, 3, 13, WPAD], BF16)
    lt12_sb = nc.alloc_sbuf_tensor("lt12_sb", [P, 44 * P], BF16)
    bias12_sb = nc.alloc_sbuf_tensor("bias12_sb", [P, 6], F32)
    bias3_sb = nc.alloc_sbuf_tensor("bias3_sb", [P, 1], F32)
    bias4_sb = nc.alloc_sbuf_tensor("bias4_sb", [P, 1], F32)
    lt3_sb = nc.alloc_sbuf_tensor("lt3_sb", [77, 2, 64], BF16)
    lt4_sb = nc.alloc_sbuf_tensor("lt4_sb", [120, P], BF16)

    PLE = NT * WPAD          # plane partition stride, elements
    PK4E = NB * W            # pk4 partition stride
    R3E = 3 * 13 * WPAD      # rhs3 partition stride
    R3PAR = [0, 1, 0, 1, 2]  # rhs3 buffer slot per tile (t4 has its own)
    # out_dram channel index per LOW7 plane
    LOWCH = [IDX31[nm] for nm in LOW7]

    def plane_out(nm, t):
        """DMA canonical rows of plane tile t to its output channel."""
        ch = IDX31[nm]
        rows = 96 if t == NT - 1 else S
        src = planes[nm][HALO : HALO + rows, t, 2:514]
        dst = _sub_ap(
            out_dram.ap(), [[W, rows], [1, W]], ch * H * W + S * t * W
        )
        nc.scalar.dma_start(out=dst, in_=src)

    with tile.TileContext(nc) as tc:
        with (
            tc.tile_pool(name="io", bufs=3) as io_pool,
            tc.tile_pool(name="front", bufs=2) as fr_pool,
            tc.tile_pool(name="psum", bufs=3, space="PSUM") as psum_pool,
            tc.tile_pool(name="psum3", bufs=2, space="PSUM") as psum3_pool,
            tc.tile_pool(name="psum4", bufs=3, space="PSUM") as psum4_pool,
            tc.tile_pool(name="c4st", bufs=4) as c4_pool,
        ):
            for _it in range(loop_k):
                # ---- parameter loads ----
                nc.sync.dma_start(out=lt12_sb[:], in_=lt12_dram[:])
                nc.sync.dma_start(out=bias12_sb[:], in_=bias12_dram[:])
                nc.sync.dma_start(out=bias3_sb[:], in_=bias3_dram[:])
                nc.sync.dma_start(out=bias4_sb[:], in_=bias4_dram[:])
                nc.sync.dma_start(out=lt3_sb[:], in_=lt3_dram[:])
                nc.sync.dma_start(out=lt4_sb[:], in_=lt4_dram[:])

                # ---- one-time zeroing: W margins + bottom tails ----
                for nm in PLANE_NAMES:
                    for t in range(NT):
                        nc.gpsimd.memset(planes[nm][:, t, 0:2], 0.0)
                        nc.gpsimd.memset(planes[nm][:, t, 514:516], 0.0)
                    # tail rows of last tile (drains only write [0:108])
                    nc.gpsimd.memset(planes[nm][96:P, NT - 1, :], 0.0)

                # ---- front-end: sq = wrap(diff)^2 ----
                for t in range(NT):
                    p_lo = HALO if t == 0 else 0
                    p_hi = 108 if t == NT - 1 else P
                    n = p_hi - p_lo
                    r_lo = S * t - HALO + p_lo
                    A = io_pool.tile([P, W], F32, tag="A")
                    B = io_pool.tile([P, W], F32, tag="B")
                    if t == 0:
                        nc.gpsimd.memset(A[0:32, :], 0.0)
                        nc.gpsimd.memset(B[0:32, :], 0.0)
                    if t == NT - 1:
                        nc.gpsimd.memset(A[96:P, :], 0.0)
                        nc.gpsimd.memset(B[96:P, :], 0.0)
                    nc.sync.dma_start(
                        out=A[p_lo:p_hi, :], in_=p_dram[r_lo : r_lo + n, :]
                    )
                    if t == 0:
                        nc.sync.dma_start(
                            out=B[p_lo + 1 : p_hi, :], in_=p_dram[0 : n - 1, :]
                        )
                        nc.sync.dma_start(
                            out=B[p_lo : p_lo + 1, :], in_=p_dram[0:1, :]
                        )
                    else:
                        nc.sync.dma_start(
                            out=B[p_lo:p_hi, :],
                            in_=p_dram[r_lo - 1 : r_lo - 1 + n, :],
                        )
                    V = fr_pool.tile([P, W], F32, tag="V")
                    K1 = fr_pool.tile([P, W], F32, tag="K1")
                    K2 = fr_pool.tile([P, W], F32, tag="K2")
                    K3 = fr_pool.tile([P, W], F32, tag="K3")
                    K4 = fr_pool.tile([P, W], F32, tag="K4")
                    nc.vector.tensor_tensor(V[:], A[:], B[:], ao.subtract)
                    nc.vector.tensor_scalar(K1[:], V[:], PI, None, ao.is_ge)
                    nc.vector.tensor_scalar(K2[:], V[:], 3 * PI, None, ao.is_ge)
                    nc.vector.tensor_scalar(K3[:], V[:], -PI, None, ao.is_le)
                    nc.vector.tensor_scalar(K4[:], V[:], -3 * PI, None, ao.is_le)
                    nc.vector.tensor_tensor(K1[:], K1[:], K2[:], ao.add)
                    nc.vector.tensor_tensor(K3[:], K3[:], K4[:], ao.add)
                    nc.vector.tensor_tensor(K1[:], K1[:], K3[:], ao.subtract)
                    nc.vector.scalar_tensor_tensor(
                        V[:], K1[:], -2 * PI, V[:], ao.mult, ao.add
                    )
                    sq_dst = planes["sq"][:, t, 2:514]
                    nc.vector.tensor_tensor(sq_dst, V[:], V[:], ao.mult)
                    plane_out("sq", t)

                # ---- conv1, conv2: banded matmuls, host-built lhsT ----
                jband = 0
                bias_col = 0
                for c in range(2):
                    dil, pad_top, pad_left, KH, KW = CONV_GEOM[c]
                    in_names = CONV_INPUTS[c]
                    O = CONV_OUT[c]
                    for o in range(O):
                        bands = []
                        coffs = []
                        srcs = []
                        for ci, nm in enumerate(in_names):
                            for kw in range(KW):
                                bands.append(
                                    lt12_sb[:, jband * P : (jband + 1) * P]
                                )
                                jband += 1
                                coffs.append(2 + kw * dil - pad_left)
                                srcs.append(nm)
                        out_nm = (
                            ["c1_0", "c1_1"][o] if c == 0 else f"c2_{o}"
                        )
                        bias_ap = bias12_sb[:, bias_col + o : bias_col + o + 1]
                        for t in range(NT):
                            ps = psum_pool.tile([P, W], F32, tag="ps")
                            for i, band in enumerate(bands):
                                rhs = _sub_ap(
                                    planes[srcs[i]].ap(),
                                    [[PLE, P], [1, W]],
                                    t * WPAD + coffs[i],
                                )
                                nc.tensor.matmul(
                                    ps, band, rhs,
                                    start=(i == 0), stop=(i == len(bands) - 1),
                                )
                            p_hi = 108 if t == NT - 1 else P
                            nc.scalar.add(
                                planes[out_nm][0:p_hi, t, 2:514],
                                ps[0:p_hi, :],
                                bias_ap[0:p_hi, :],
                            )
                    # zero top halo of the new planes (rows < 0)
                    outs = ["c1_0", "c1_1"] if c == 0 else [
                        f"c2_{i}" for i in range(4)
                    ]
                    for nm in outs:
                        nc.gpsimd.memset(planes[nm][0:HALO, 0, :], 0.0)
                        for t in range(NT):
                            plane_out(nm, t)
                    bias_col += O

                # ---- rhs3 zero margins + zero edge cells (once) ----
                nc.gpsimd.memset(rhs3[0:77, :, :, 0:2], 0.0)
                nc.gpsimd.memset(rhs3[0:77, :, :, 514:516], 0.0)
                for ci in range(7):
                    # t=0, slot 0, k=0, r'=0: image row -1 -> zero
                    dst = _sub_ap(rhs3.ap(), [[R3E, 1], [1, WPAD]], 11 * ci * R3E)
                    nc.sync.dma_start(out=dst, in_=zeros_dram[0:1, :])
                    # t=4, slot 2, k=11, r' in {9,10}: rows 512,513 -> zero
                    dst = _sub_ap(
                        rhs3.ap(),
                        [[R3E, 2], [1, WPAD]],
                        (11 * ci + 9) * R3E + 2 * 13 * WPAD + 11 * WPAD,
                    )
                    nc.sync.dma_start(out=dst, in_=zeros_dram[:])

                # ---- conv3: packed contraction, 8-row blocks ----
                for t in range(NT):
                    nk = NKT[t]
                    par = R3PAR[t]
                    for ci in range(7):
                        ch = LOWCH[ci]
                        base = 11 * ci * R3E + par * 13 * WPAD
                        if t == 0:
                            # r' 1..10 for all k
                            src = _sub_ap(
                                out_dram.ap(),
                                [[W, 10], [8 * W, nk], [1, W]],
                                ch * H * W,
                            )
                            dst = _sub_ap(
                                rhs3.ap(),
                                [[R3E, 10], [WPAD, nk], [1, W]],
                                base + R3E + 2,
                            )
                            nc.scalar.dma_start(out=dst, in_=src)
                            # r'=0 (row 8k-1) for k 1..12
                            src = _sub_ap(
                                out_dram.ap(),
                                [[8 * W, 12], [1, W]],
                                ch * H * W + 7 * W,
                            )
                            dst = _sub_ap(
                                rhs3.ap(),
                                [[R3E, 1], [WPAD, 12], [1, W]],
                                base + WPAD + 2,
                            )
                            nc.scalar.dma_start(out=dst, in_=src)
                        elif t == NT - 1:
                            # k 0..10 all r'; k=11 r' 0..8
                            src = _sub_ap(
                                out_dram.ap(),
                                [[W, 11], [8 * W, 11], [1, W]],
                                ch * H * W + (S * t - 1) * W,
                            )
                            dst = _sub_ap(
                                rhs3.ap(),
                                [[R3E, 11], [WPAD, 11], [1, W]],
                                base + 2,
                            )
                            nc.scalar.dma_start(out=dst, in_=src)
                            src = _sub_ap(
                                out_dram.ap(),
                                [[W, 9], [1, W]],
                                ch * H * W + (S * t + 87) * W,
                            )
                            dst = _sub_ap(
                                rhs3.ap(),
                                [[R3E, 9], [1, W]],
                                base + 11 * WPAD + 2,
                            )
                            nc.scalar.dma_start(out=dst, in_=src)
                        else:
                            src = _sub_ap(
                                out_dram.ap(),
                                [[W, 11], [8 * W, nk], [1, W]],
                                ch * H * W + (S * t - 1) * W,
                            )
                            dst = _sub_ap(
                                rhs3.ap(),
                                [[R3E, 11], [WPAD, nk], [1, W]],
                                base + 2,
                            )
                            nc.scalar.dma_start(out=dst, in_=src)
                    for k in range(nk):
                        b = 13 * t + k
                        ps3 = psum3_pool.tile([64, W], F32, tag="ps3")
                        for kw in range(2):
                            rhs = _sub_ap(
                                rhs3.ap(),
                                [[R3E, 77], [1, W]],
                                par * 13 * WPAD + k * WPAD + 1 + 3 * kw,
                            )
                            nc.tensor.matmul(
                                ps3,
                                lt3_sb[:, kw, :],
                                rhs,
                                start=(kw == 0),
                                stop=(kw == 1),
                            )
                        nc.vector.tensor_scalar(
                            pk4[0:64, b, :], ps3[0:64, :],
                            bias3_sb[0:64, :], None, ao.add,
                        )

                # ---- conv4: low7 part of pk4 from output channels ----
                for ci7 in range(7):
                    ch = LOWCH[ci7]
                    src = _sub_ap(
                        out_dram.ap(),
                        [[W, 8], [8 * W, NB], [1, W]],
                        ch * H * W,
                    )
                    nc.sync.dma_start(
                        out=pk4[64 + 8 * ci7 : 72 + 8 * ci7, :, :], in_=src
                    )
                for b in range(NB):
                    ps4 = psum4_pool.tile([P, W], F32, tag="ps4")
                    nc.tensor.matmul(
                        ps4, lt4_sb[:], pk4[0:120, b, :], start=True, stop=True
                    )
                    st = c4_pool.tile([P, W], BF16, tag="c4")
                    nc.vector.tensor_scalar(
                        st[:], ps4[:], bias4_sb[:], None, ao.add
                    )
                    eng = [nc.sync, nc.scalar][b % 2]
                    dst = _sub_ap(
                        out_dram.ap(),
                        [[H * W, 16], [W, 8], [1, W]],
                        8 * b * W,
                    )
                    eng.dma_start(out=dst, in_=st[:])

                # ---- DMA out: c3 channels from pk4 ----
                for o in range(8):
                    ch = 16 + o
                    dst = _sub_ap(
                        out_dram.ap(),
                        [[W, 8], [8 * W, NB], [1, W]],
                        ch * H * W,
                    )
                    nc.sync.dma_start(out=dst, in_=pk4[8 * o : 8 * o + 8, :, :])

    nc.compile()
    return nc


_NC_CACHE = None


def _get_nc():
    global _NC_CACHE
    if _NC_CACHE is None:
        _NC_CACHE = build_nc()
    return _NC_CACHE


def _expand(out31):
    """[31,H,W] bf16/f32 -> [48,H,W] f32 with duplicated concat channels."""
    out31 = np.asarray(out31).astype(np.float32)
    full = np.empty((48, H, W), np.float32)
    for ch, nm in enumerate(FULL_CH):
        full[ch] = out31[IDX31[nm]]
    return full


def _run(inputs, trace=False):
    inputs = {k: np.asarray(v) for k, v in inputs.items()}
    nc = _get_nc()
    tabs = _host_tables(inputs)
    feat = inputs["feature_in"].astype(np.float32)
    n_cores = feat.shape[0]
    in_maps = [
        {
            "p": feat[b, 0],
            "lhsT12": tabs["lhsT12"],
            "bias12": tabs["bias12"],
            "bias3": tabs["bias3"],
            "bias4": tabs["bias4"],
            "lhsT3": tabs["lhsT3"],
            "lhsT4": tabs["lhsT4"],
            "zeros": np.zeros((2, WPAD), BF16_NP),
        }
        for b in range(n_cores)
    ]
    res = bass_utils.run_bass_kernel_spmd(
        nc, in_maps, core_ids=list(range(n_cores)), trace=trace
    )
    out = np.stack(
        [_expand(res.results[b]["out"]) for b in range(n_cores)], axis=0
    )
    return out, res


def kernel(**inputs):
    return _run(inputs, trace=False)[0]
